# revision 17
# baseline (speedup 1.0000x reference)
"""Trainium2 Bass kernel: Brownian motion on O(3) via ambient SDE steps.

Math: each reference step is
    inc = sqrt(dt) * eps
    v   = 0.5*(inc - x inc^T x) = x @ Omega,  Omega = 0.5*(A - A^T), A = x^T inc
    x'  = polar(x + v) = x @ polar(I + Omega)
and for a 3x3 skew Omega with axis vector w (|w| = theta):
    polar(I + Omega) = Q = alpha*I + Omega(alpha*w) + beta * w w^T
    c = sqrt(1 + theta^2), alpha = 1/c, beta = 1/(c*(c+1))
which matches the SVD projection to machine precision (no SVD needed).

The per-sample scale sd2 = 0.5*sqrt(t/steps) is folded into the state:
    Xs = sd2 * x   =>   w = axial(Xs^T N) needs no per-step scaling, and
    Xs' = Xs @ Q keeps the fold;  x_final = Xs_final / sd2  (once).

Two decoupled cohorts per core (samples split along the free dim):
  - DVE cohort (fp16 SoA planes, 2x_1p mode): products/assembly/xQ as
    tensor_tensor; theta^2 and alpha = rsqrt(1+theta^2) via three custom
    DVE ops (sq(a)+sq(b); sq(a)+b+1; linear-seed + one Newton step fused,
    max rel err 2.7e-4 on theta^2 in [0, 0.9]); beta(alpha) deg-1 via one
    fused tensor_scalar.
  - GPSIMD cohort (fp32): reads the raw AoS f32 noise directly (Pool has
    no stride-1 perf mode, so AoS strides are free) - no ScalarE
    conversion, no cross-engine deps at all; alpha poly deg-3 via fused
    immediate tensor_scalar ops (Pool-legal, and 0.6 ISA efficiency vs
    0.42 for tensor_tensor).
ScalarE only converts the DVE cohort's noise (AoS f32 -> SoA f16) and
does one-time init/final layout conversions. Steady state has no
cross-engine round trips inside a step, so both engines run at their
own throughput; the sample split balances them.

Sharding: pure data parallel over the batch across 8 NeuronCores.
"""

import os
import sys

import numpy as np

for _p in ("/opt/trn_rl_repo",):
    if _p not in sys.path and os.path.isdir(_p):
        sys.path.insert(0, _p)

import concourse.bass as bass
import concourse.dve_ops as dve_ops
import concourse.tile as tile
from concourse import bacc, mybir
from concourse.bass_utils import run_bass_kernel_spmd
from concourse.dve_ops import DveOp
from concourse.dve_spec import C0, C1, C2, One, Spec, Src0, Src1, lower, sq
from concourse.dve_table_gen import dve_ver_for
from concourse.dve_uop import DveOpSpec

AF = mybir.ActivationFunctionType
OP = mybir.AluOpType
F32 = mybir.dt.float32
F16 = mybir.dt.float16

B = 262144
NCORES = 8
BL = B // NCORES          # 32768 samples per core
P = 128
STEPS = 20

# samples per partition handled by GPSIMD (rest on DVE)
SB_GPSIMD = 49

# alpha = rsqrt(1 + theta^2): linear seed + one fused Newton step, constants
# minimax-optimized jointly on u1 = 1+theta^2 in [1, 1.9] (rel err 2.74e-4)
ALPHA_NR = (1.26296369, -0.29519499, 0.52064811)
# beta as a degree-1 polynomial OF ALPHA (beta = a^2/(1+a)); the ~5e-3 fit
# error is damped by theta^2 in Q's rank-1 term
BETA_A_POLY = [-0.20798077392841205, 0.705238169782092]
# GPSIMD cohort: alpha(u) deg-3 minimax fit on [0, 0.9] (3.5e-4), evaluated
# in Estrin form so the two halves overlap (fewer serial-dependency gaps)
ALPHA_POLY_B = [0.99964635, -0.48630541, 0.28591126, -0.09374474]


def _register_op(name, spec, rd1: bool) -> DveOp:
    """Register a custom DVE op (idempotent across re-imports)."""
    for op in dve_ops.OPS:
        if op.name == name:
            return op
    ver = dve_ver_for("TRN2")
    row = dve_ops._CUSTOM_DVE_ROW_BASE + len(dve_ops.OPS)
    sha = DveOpSpec(name=name, opcode=row, uops=lower(spec, ver=ver),
                    rd1_en=rd1).sha(ver)
    op = DveOp(name, spec, subdim=False, uops_sha={ver: sha})
    dve_ops.OPS.append(op)
    dve_ops._SUB_OPCODE_FOR_NAME[name] = row
    dve_ops.CUSTOM_DVE_SPECS[name] = spec
    return op


# th2 partial: out = w0^2 + w1^2
SQSQ = _register_op("BM_SQSQ", Spec(
    body=sq(Src0) + sq(Src1),
    reference=lambda in0, in1, s0, s1, imm2: (
        in0.astype(np.float32) ** 2 + in1.astype(np.float32) ** 2),
), rd1=True)
# u1 = 1 + w2^2 + partial  (= 1 + theta^2)
SQADD1P = _register_op("BM_SQADD1P", Spec(
    body=sq(Src0) + Src1 + One,
    reference=lambda in0, in1, s0, s1, imm2: (
        in0.astype(np.float32) ** 2 + in1.astype(np.float32) + 1.0),
), rd1=True)


def _alpha_ref(in0, in1, s0, s1, imm2):
    x = in0.astype(np.float32)
    a0 = x * s1 + s0
    return a0 * (1.0 + imm2 * (1.0 - a0 * a0 * x))


# alpha = rsqrt(u1): a0 = C0 + C1*u1 (seed), out = a0*(1 + C2*(1 - u1*a0^2))
_a0 = Src0 * C1 + C0
ALPHA_FULL = _register_op("BM_ALPHA_FULL", Spec(
    body=_a0 * (One + C2 * (One - sq(_a0) * Src0)),
    reference=_alpha_ref,
), rd1=False)


def _step_common(eng, Sh, h, xv2, nv2, f16: bool):
    """Products -> w -> (theta^2 path is per-engine) shared plumbing.
    Emits products + skew-difference + reduction into h["W"]."""
    ppn = h["PPN"]
    ppv = ppn[:, 0:9 * Sh].rearrange("p (c r s) -> p c r s", c=3, r=3)
    pnv = ppn[:, 9 * Sh:].rearrange("p (c r s) -> p c r s", c=3, r=3)
    gv = ppn[:].rearrange("p (g r s) -> p g r s", g=6, r=3)
    # merged product instructions (3 instead of 6)
    eng.tensor_tensor(ppv[:, 1:3], xv2[:, 0:2], nv2[:, 2::-2], OP.mult)
    eng.tensor_tensor(pnv[:, 0:2], xv2[:, 1:3], nv2[:, 2::-2], OP.mult)
    eng.tensor_tensor(gv[:, 0:6:5], xv2[:, 2::-2],
                      nv2[:, 1:2].broadcast_to((P, 2, 3, Sh)), OP.mult)
    # w_c = sum_r (PP - PN)   (sd2 already folded into X)
    w3 = h["W"][:].rearrange("p (c s) -> p c s", c=3)
    ws3 = h["WS"][:].rearrange("p (c s) -> p c s", c=3)
    eng.tensor_tensor(ppn[:, 0:9 * Sh], ppn[:, 0:9 * Sh], ppn[:, 9 * Sh:],
                      OP.subtract)
    eng.tensor_tensor(ws3, ppv[:, :, 0], ppv[:, :, 1], OP.add)
    eng.tensor_tensor(w3, ws3, ppv[:, :, 2], OP.add)
    return w3


def _step_q_and_xq(eng, Sh, h, w3, xv2, Xn):
    """Q assembly from (w, alpha, beta planes in h["AB"]) and Xn = Xc @ Q."""
    ab2 = h["AB"][:].rearrange("p (g s) -> p g s", g=2)
    # [WP; WB] = [alpha*w ; beta*w] in one instruction
    wpb = h["WPB"][:].rearrange("p (g c s) -> p g c s", g=2, c=3)
    eng.tensor_tensor(
        wpb,
        w3.unsqueeze(1).broadcast_to((P, 2, 3, Sh)),
        ab2.unsqueeze(2).broadcast_to((P, 2, 3, Sh)),
        OP.mult)
    wpv = h["WPB"][:, 0:3 * Sh].rearrange("p (c s) -> p c s", c=3)
    wb3 = h["WPB"][:, 3 * Sh:].rearrange("p (c s) -> p c s", c=3)
    # Q = (beta*w) (x) w  +  alpha*I  +  skew(alpha*w)
    qv9 = h["QT"][:].rearrange("p (e s) -> p e s", e=9)
    qve = h["QT"][:].rearrange("p (a b s) -> p a b s", a=3, b=3)
    eng.tensor_tensor(qve,
                      wb3.unsqueeze(2).broadcast_to((P, 3, 3, Sh)),
                      w3.unsqueeze(1).broadcast_to((P, 3, 3, Sh)),
                      OP.mult)
    albc = h["AB"][:, 0:Sh].unsqueeze(1).broadcast_to((P, 3, Sh))
    eng.tensor_tensor(qv9[:, 0:9:4], qv9[:, 0:9:4], albc, OP.add)
    eng.tensor_tensor(qv9[:, 2:4], qv9[:, 2:4], wpv[:, 1:3], OP.add)
    eng.tensor_tensor(qv9[:, 7:8], qv9[:, 7:8], wpv[:, 0:1], OP.add)
    eng.tensor_tensor(qv9[:, 5:7], qv9[:, 5:7], wpv[:, 0:2], OP.subtract)
    eng.tensor_tensor(qv9[:, 1:2], qv9[:, 1:2], wpv[:, 2:3], OP.subtract)
    # Xn = Xc @ Q
    qv = h["QT"][:].rearrange("p (cc j s) -> p cc j s", cc=3, j=3)
    tbf = h["TBIG"]
    for cc in range(3):
        tv = tbf[:, cc * 9 * Sh:(cc + 1) * 9 * Sh].rearrange(
            "p (rr j s) -> p rr j s", rr=3, j=3)
        eng.tensor_tensor(
            tv,
            xv2[:, cc].unsqueeze(2).broadcast_to((P, 3, 3, Sh)),
            qv[:, cc].unsqueeze(1).broadcast_to((P, 3, 3, Sh)),
            OP.mult)
    eng.tensor_tensor(Xn[:], tbf[:, 0:9 * Sh], tbf[:, 9 * Sh:18 * Sh], OP.add)
    eng.tensor_tensor(Xn[:], Xn[:], tbf[:, 18 * Sh:], OP.add)


def build_nc(bl: int = BL, steps: int = STEPS, sb: int = SB_GPSIMD) -> bass.Bass:
    S = bl // P               # samples per partition
    F9 = 9 * S
    if sb * 2 >= S:
        sb = (S // 4) & ~1    # keep the split sane for small test sizes
    sd = S - sb               # DVE cohort width

    a0_, a1_, a2, a3 = ALPHA_POLY_B
    b1, b0 = BETA_A_POLY[1], BETA_A_POLY[0]
    nr_c0, nr_c1, nr_c2 = ALPHA_NR

    nc = bacc.Bacc("TRN2", target_bir_lowering=False, debug=False)
    with tile.TileContext(nc) as tc:
        x_d = nc.dram_tensor("x", [bl, 3, 3], F32, kind="ExternalInput")
        t_d = nc.dram_tensor("t", [bl, 1], F32, kind="ExternalInput")
        n_d = nc.dram_tensor("noise", [steps, bl, 3, 3], F32,
                             kind="ExternalInput")
        o_d = nc.dram_tensor("out", [bl, 3, 3], F32, kind="ExternalOutput")

        xr = x_d.rearrange("(p s) a b -> p (s a b)", p=P)
        tr = t_d.rearrange("(p s) o -> p (s o)", p=P)
        nr = n_d.rearrange("k (p s) a b -> k p (s a b)", p=P)
        orr = o_d.rearrange("(p s) a b -> p (s a b)", p=P)

        with (
            tc.tile_pool(name="state", bufs=1) as pool,
            tc.tile_pool(name="nzf", bufs=4) as nzfpool,
            tc.tile_pool(name="nza", bufs=3) as nzapool,
        ):
            XIN = pool.tile([P, F9], F32, name="XIN", tag="XIN")
            Tt = pool.tile([P, S], F32, name="Tt", tag="Tt")
            SD2H = pool.tile([P, S], F16, name="SD2H", tag="SD2H")
            SD2F = pool.tile([P, S], F32, name="SD2F", tag="SD2F")
            INVF = pool.tile([P, S], F32, name="INVF", tag="INVF")
            INVH = pool.tile([P, sd], F16, name="INVH", tag="INVH")
            OUTF = pool.tile([P, F9], F32, name="OUTF", tag="OUTF")

            # t first (tiny, unblocks sd2), then x, then the noise stream -
            # all on the same queue so the state DMAs are not stuck behind
            # noise prefetches
            nc.sync.dma_start(Tt[:], tr)
            NZF0 = nzfpool.tile([P, F9], F32, name="NZF", tag="NZF")
            nc.sync.dma_start(NZF0[:], nr[0])
            nc.sync.dma_start(XIN[:, 0:9 * sd], xr[:, 0:9 * sd])
            nc.sync.dma_start(XIN[:, 9 * sd:], xr[:, 9 * sd:])
            # sd2 = 0.5*sqrt(t/steps) = sqrt(t/(4*steps))
            nc.scalar.activation(SD2H[:], Tt[:], AF.Sqrt, bias=0.0,
                                 scale=1.0 / (4.0 * steps))
            nc.scalar.activation(SD2F[:], Tt[:], AF.Sqrt, bias=0.0,
                                 scale=1.0 / (4.0 * steps))
            # 1/sd2 via the ~51-ULP fast reciprocal; emitted first in the DVE
            # program so it runs during the otherwise-idle ramp
            nc.vector.reciprocal_approx_fast(INVF[:], SD2F[:])

            # --- DVE cohort tiles (f16) ---
            ha = {
                "X": [pool.tile([P, 9 * sd], F16, name="XAa", tag="XAa"),
                      pool.tile([P, 9 * sd], F16, name="XAb", tag="XAb")],
                "PPN": pool.tile([P, 18 * sd], F16, name="PPNA", tag="PPNA"),
                "W": pool.tile([P, 3 * sd], F16, name="WA", tag="WA"),
                "WS": pool.tile([P, 3 * sd], F16, name="WSA", tag="WSA"),
                "TH2P": pool.tile([P, sd], F32, name="TH2PA", tag="TH2PA"),
                "U1": pool.tile([P, sd], F32, name="U1A", tag="U1A"),
                "AB": pool.tile([P, 2 * sd], F16, name="ABA", tag="ABA"),
                "WPB": pool.tile([P, 6 * sd], F16, name="WPBA", tag="WPBA"),
                "QT": pool.tile([P, 9 * sd], F16, name="QTA", tag="QTA"),
                "TBIG": pool.tile([P, 27 * sd], F16, name="TBIGA",
                                  tag="TBIGA"),
            }
            # --- GPSIMD cohort tiles (f32) ---
            hb = {
                "X": [pool.tile([P, 9 * sb], F32, name="XBa", tag="XBa"),
                      pool.tile([P, 9 * sb], F32, name="XBb", tag="XBb")],
                "PPN": pool.tile([P, 18 * sb], F32, name="PPNB", tag="PPNB"),
                "W": pool.tile([P, 3 * sb], F32, name="WB_", tag="WB_"),
                "WS": pool.tile([P, 3 * sb], F32, name="WSB", tag="WSB"),
                "P2": pool.tile([P, 3 * sb], F32, name="P2B", tag="P2B"),
                "TH2": pool.tile([P, sb], F32, name="TH2B", tag="TH2B"),
                "PH": pool.tile([P, sb], F32, name="PHB", tag="PHB"),
                "PH2": pool.tile([P, sb], F32, name="PHB2", tag="PHB2"),
                "U2": pool.tile([P, sb], F32, name="U2B", tag="U2B"),
                "AB": pool.tile([P, 2 * sb], F32, name="ABB", tag="ABB"),
                "WPB": pool.tile([P, 6 * sb], F32, name="WPBB", tag="WPBB"),
                "QT": pool.tile([P, 9 * sb], F32, name="QTB", tag="QTB"),
                "TBIG": pool.tile([P, 27 * sb], F32, name="TBIGB",
                                  tag="TBIGB"),
            }

            # initial states: X0 = sd2 * x straight from the AoS f32 input
            # (single TT per cohort; AoS strides cost nothing one-time)
            xin_va = XIN[:, 0:9 * sd].rearrange("p (s e) -> p e s", e=9)
            nc.vector.tensor_tensor(
                ha["X"][0][:].rearrange("p (e s) -> p e s", e=9),
                xin_va,
                SD2H[:, 0:sd].unsqueeze(1).broadcast_to((P, 9, sd)), OP.mult)
            xin_vb = XIN[:, 9 * sd:].rearrange("p (s e) -> p e s", e=9)
            nc.gpsimd.tensor_tensor(
                hb["X"][0][:].rearrange("p (e s) -> p e s", e=9),
                xin_vb,
                SD2F[:, sd:].unsqueeze(1).broadcast_to((P, 9, sb)), OP.mult)

            for k in range(steps):
                if k == 0:
                    NZF = NZF0
                else:
                    NZF = nzfpool.tile([P, F9], F32, name="NZF", tag="NZF")
                    nc.sync.dma_start(NZF[:], nr[k])
                # DVE cohort noise: AoS f32 -> SoA f16 (ScalarE)
                NZA = nzapool.tile([P, 9 * sd], F16, name="NZA", tag="NZA")
                nzf_va = NZF[:, 0:9 * sd].rearrange("p (s e) -> p e s", e=9)
                nc.scalar.copy(NZA[:].rearrange("p (e s) -> p e s", e=9),
                               nzf_va)

                # ---------- DVE cohort ----------
                eng = nc.vector
                Xc, Xn = ha["X"][k % 2], ha["X"][(k + 1) % 2]
                xv2 = Xc[:].rearrange("p (rr e s) -> p e rr s", rr=3, e=3)
                nv2 = NZA[:].rearrange("p (rr e s) -> p e rr s", rr=3, e=3)
                w3 = _step_common(eng, sd, ha, xv2, nv2, True)
                # theta^2 and alpha via fused custom ops, beta via one TSP
                eng._custom_dve(SQSQ, out=ha["TH2P"][:],
                                in0=ha["W"][:, 0:sd], in1=ha["W"][:, sd:2 * sd])
                eng._custom_dve(SQADD1P, out=ha["U1"][:],
                                in0=ha["W"][:, 2 * sd:], in1=ha["TH2P"][:])
                eng._custom_dve(ALPHA_FULL, out=ha["AB"][:, 0:sd],
                                in0=ha["U1"][:], s0=nr_c0, s1=nr_c1,
                                imm2=nr_c2)
                eng.tensor_scalar(ha["AB"][:, sd:], ha["AB"][:, 0:sd],
                                  float(b1), float(b0), OP.mult, OP.add)
                _step_q_and_xq(eng, sd, ha, w3, xv2, Xn)

                # ---------- GPSIMD cohort (f32, raw AoS noise) ----------
                eng = nc.gpsimd
                Xc, Xn = hb["X"][k % 2], hb["X"][(k + 1) % 2]
                xv2b = Xc[:].rearrange("p (rr e s) -> p e rr s", rr=3, e=3)
                nv2b = NZF[:, 9 * sd:].rearrange("p (s rr e) -> p e rr s",
                                                 rr=3, e=3)
                w3b = _step_common(eng, sb, hb, xv2b, nv2b, False)
                # theta^2
                p2v = hb["P2"][:].rearrange("p (c s) -> p c s", c=3)
                eng.tensor_tensor(hb["P2"][:], hb["W"][:], hb["W"][:],
                                  OP.mult)
                eng.tensor_tensor(hb["TH2"][:], p2v[:, 0], p2v[:, 1], OP.add)
                eng.tensor_tensor(hb["TH2"][:], hb["TH2"][:], p2v[:, 2],
                                  OP.add)
                # alpha deg-3 in Estrin form (three leading ops independent)
                eng.tensor_scalar(hb["PH"][:], hb["TH2"][:], float(a1_),
                                  float(a0_), OP.mult, OP.add)
                eng.tensor_scalar(hb["PH2"][:], hb["TH2"][:], float(a3),
                                  float(a2), OP.mult, OP.add)
                eng.tensor_tensor(hb["U2"][:], hb["TH2"][:], hb["TH2"][:],
                                  OP.mult)
                eng.tensor_tensor(hb["PH2"][:], hb["PH2"][:], hb["U2"][:],
                                  OP.mult)
                eng.tensor_tensor(hb["AB"][:, 0:sb], hb["PH"][:], hb["PH2"][:],
                                  OP.add)
                eng.tensor_scalar(hb["AB"][:, sb:], hb["AB"][:, 0:sb],
                                  float(b1), float(b0), OP.mult, OP.add)
                _step_q_and_xq(eng, sb, hb, w3b, xv2b, Xn)

            # final: unscale (x = Xs / sd2), convert to AoS f32, DMA out
            nc.scalar.copy(INVH[:], INVF[:, 0:sd])
            xfa = ha["X"][steps % 2]
            XOUT = pool.tile([P, 9 * sd], F16, name="XOUT", tag="XOUT")
            sd_3 = (sd // 3) & ~1
            for c0, c1 in ((0, sd_3), (sd_3, 2 * sd_3), (2 * sd_3, sd)):
                cw = c1 - c0
                nc.vector.tensor_tensor(
                    XOUT[:, 9 * c0:9 * c1].rearrange("p (e s) -> p e s", e=9),
                    xfa[:].rearrange("p (e s) -> p e s", e=9)[:, :, c0:c1],
                    INVH[:, c0:c1].unsqueeze(1).broadcast_to((P, 9, cw)),
                    OP.mult)
                of_va = OUTF[:, 9 * c0:9 * c1].rearrange(
                    "p (s e) -> p s e", e=9)
                nc.scalar.copy(of_va, XOUT[:, 9 * c0:9 * c1].rearrange(
                    "p (e s) -> p s e", e=9))
                nc.scalar.dma_start(orr[:, 9 * c0:9 * c1],
                                    OUTF[:, 9 * c0:9 * c1])
            # GPSIMD cohort writes its AoS f32 slice directly
            xfb = hb["X"][steps % 2]
            of_vb = OUTF[:, 9 * sd:].rearrange("p (s e) -> p e s", e=9)
            nc.gpsimd.tensor_tensor(
                of_vb,
                xfb[:].rearrange("p (e s) -> p e s", e=9),
                INVF[:, sd:].unsqueeze(1).broadcast_to((P, 9, sb)), OP.mult)
            nc.sync.dma_start(orr[:, 9 * sd:], OUTF[:, 9 * sd:])
    nc.compile()
    return nc


_NC_CACHE = {}


def _get_nc(bl: int, steps: int) -> bass.Bass:
    key = (bl, steps)
    if key not in _NC_CACHE:
        _NC_CACHE[key] = build_nc(bl, steps)
    return _NC_CACHE[key]


last_exec_time_ns = None
last_results = None


def kernel(x: np.ndarray, t: np.ndarray, noise: np.ndarray, steps=STEPS,
           _trace: bool = False, **_unused) -> np.ndarray:
    global last_exec_time_ns, last_results
    steps = int(steps)
    b = x.shape[0]
    assert b % NCORES == 0
    bl = b // NCORES
    assert bl % P == 0

    x = np.ascontiguousarray(np.asarray(x, dtype=np.float32))
    t = np.ascontiguousarray(np.asarray(t, dtype=np.float32))
    noise = np.ascontiguousarray(np.asarray(noise, dtype=np.float32))

    nc = _get_nc(bl, steps)
    in_maps = []
    for i in range(NCORES):
        sl = slice(i * bl, (i + 1) * bl)
        in_maps.append({
            "x": x[sl],
            "t": t[sl],
            "noise": np.ascontiguousarray(noise[:, sl]),
        })
    res = run_bass_kernel_spmd(
        nc, in_maps, core_ids=list(range(NCORES)), trace=_trace)
    last_exec_time_ns = res.exec_time_ns
    last_results = res
    out = np.concatenate([r["out"] for r in res.results], axis=0)
    return out.astype(np.float32)


# revision 20
# speedup vs baseline: 1.0039x; 1.0039x over previous
"""Trainium2 Bass kernel: Brownian motion on O(3) via ambient SDE steps.

Math: each reference step is
    inc = sqrt(dt) * eps
    v   = 0.5*(inc - x inc^T x) = x @ Omega,  Omega = 0.5*(A - A^T), A = x^T inc
    x'  = polar(x + v) = x @ polar(I + Omega)
and for a 3x3 skew Omega with axis vector w (|w| = theta):
    polar(I + Omega) = Q = alpha*I + Omega(alpha*w) + beta * w w^T
    c = sqrt(1 + theta^2), alpha = 1/c, beta = 1/(c*(c+1))
which matches the SVD projection to machine precision (no SVD needed).

The per-sample scale sd2 = 0.5*sqrt(t/steps) is folded into the state:
    Xs = sd2 * x   =>   w = axial(Xs^T N) needs no per-step scaling, and
    Xs' = Xs @ Q keeps the fold;  x_final = Xs_final / sd2  (once).

Two decoupled cohorts per core (samples split along the free dim):
  - DVE cohort (fp16 SoA planes, 2x_1p mode): products/assembly/xQ as
    tensor_tensor; theta^2 and alpha = rsqrt(1+theta^2) via three custom
    DVE ops (sq(a)+sq(b); sq(a)+b+1; linear-seed + one Newton step fused,
    max rel err 2.7e-4 on theta^2 in [0, 0.9]); beta(alpha) deg-1 via one
    fused tensor_scalar.
  - GPSIMD cohort (fp32): reads the raw AoS f32 noise directly (Pool has
    no stride-1 perf mode, so AoS strides are free) - no ScalarE
    conversion, no cross-engine deps at all; alpha poly deg-3 via fused
    immediate tensor_scalar ops (Pool-legal, and 0.6 ISA efficiency vs
    0.42 for tensor_tensor).
ScalarE only converts the DVE cohort's noise (AoS f32 -> SoA f16) and
does one-time init/final layout conversions. Steady state has no
cross-engine round trips inside a step, so both engines run at their
own throughput; the sample split balances them.

Sharding: pure data parallel over the batch across 8 NeuronCores.
"""

import os
import sys

import numpy as np

for _p in ("/opt/trn_rl_repo",):
    if _p not in sys.path and os.path.isdir(_p):
        sys.path.insert(0, _p)

import concourse.bass as bass
import concourse.dve_ops as dve_ops
import concourse.tile as tile
from concourse import bacc, mybir
from concourse.bass_utils import run_bass_kernel_spmd
from concourse.dve_ops import DveOp
from concourse.dve_spec import C0, C1, C2, One, Spec, Src0, Src1, lower, sq
from concourse.dve_table_gen import dve_ver_for
from concourse.dve_uop import DveOpSpec

AF = mybir.ActivationFunctionType
OP = mybir.AluOpType
F32 = mybir.dt.float32
F16 = mybir.dt.float16

B = 262144
NCORES = 8
BL = B // NCORES          # 32768 samples per core
P = 128
STEPS = 20

# samples per partition handled by GPSIMD (rest on DVE)
SB_GPSIMD = 49

# alpha = rsqrt(1 + theta^2): linear seed + one fused Newton step, constants
# minimax-optimized jointly on u1 = 1+theta^2 in [1, 1.9] (rel err 2.74e-4)
ALPHA_NR = (1.26296369, -0.29519499, 0.52064811)
# beta as a degree-1 polynomial OF ALPHA (beta = a^2/(1+a)); the ~5e-3 fit
# error is damped by theta^2 in Q's rank-1 term
BETA_A_POLY = [-0.20798077392841205, 0.705238169782092]
# GPSIMD cohort: alpha(u) deg-3 minimax fit on [0, 0.9] (3.5e-4), evaluated
# in Estrin form so the two halves overlap (fewer serial-dependency gaps)
ALPHA_POLY_B = [0.99964635, -0.48630541, 0.28591126, -0.09374474]


def _register_op(name, spec, rd1: bool) -> DveOp:
    """Register a custom DVE op (idempotent across re-imports)."""
    for op in dve_ops.OPS:
        if op.name == name:
            return op
    ver = dve_ver_for("TRN2")
    row = dve_ops._CUSTOM_DVE_ROW_BASE + len(dve_ops.OPS)
    sha = DveOpSpec(name=name, opcode=row, uops=lower(spec, ver=ver),
                    rd1_en=rd1).sha(ver)
    op = DveOp(name, spec, subdim=False, uops_sha={ver: sha})
    dve_ops.OPS.append(op)
    dve_ops._SUB_OPCODE_FOR_NAME[name] = row
    dve_ops.CUSTOM_DVE_SPECS[name] = spec
    return op


# th2 partial: out = w0^2 + w1^2
SQSQ = _register_op("BM_SQSQ", Spec(
    body=sq(Src0) + sq(Src1),
    reference=lambda in0, in1, s0, s1, imm2: (
        in0.astype(np.float32) ** 2 + in1.astype(np.float32) ** 2),
), rd1=True)
# u1 = 1 + w2^2 + partial  (= 1 + theta^2)
SQADD1P = _register_op("BM_SQADD1P", Spec(
    body=sq(Src0) + Src1 + One,
    reference=lambda in0, in1, s0, s1, imm2: (
        in0.astype(np.float32) ** 2 + in1.astype(np.float32) + 1.0),
), rd1=True)


def _alpha_ref(in0, in1, s0, s1, imm2):
    x = in0.astype(np.float32)
    a0 = x * s1 + s0
    return a0 * (1.0 + imm2 * (1.0 - a0 * a0 * x))


# alpha = rsqrt(u1): a0 = C0 + C1*u1 (seed), out = a0*(1 + C2*(1 - u1*a0^2))
_a0 = Src0 * C1 + C0
ALPHA_FULL = _register_op("BM_ALPHA_FULL", Spec(
    body=_a0 * (One + C2 * (One - sq(_a0) * Src0)),
    reference=_alpha_ref,
), rd1=False)


def _step_common(eng, Sh, h, xv2, nv2, f16: bool):
    """Products -> w -> (theta^2 path is per-engine) shared plumbing.
    Emits products + skew-difference + reduction into h["W"]."""
    ppn = h["PPN"]
    ppv = ppn[:, 0:9 * Sh].rearrange("p (c r s) -> p c r s", c=3, r=3)
    pnv = ppn[:, 9 * Sh:].rearrange("p (c r s) -> p c r s", c=3, r=3)
    gv = ppn[:].rearrange("p (g r s) -> p g r s", g=6, r=3)
    # merged product instructions (3 instead of 6)
    eng.tensor_tensor(ppv[:, 1:3], xv2[:, 0:2], nv2[:, 2::-2], OP.mult)
    eng.tensor_tensor(pnv[:, 0:2], xv2[:, 1:3], nv2[:, 2::-2], OP.mult)
    eng.tensor_tensor(gv[:, 0:6:5], xv2[:, 2::-2],
                      nv2[:, 1:2].broadcast_to((P, 2, 3, Sh)), OP.mult)
    # w_c = sum_r (PP - PN)   (sd2 already folded into X)
    w3 = h["W"][:].rearrange("p (c s) -> p c s", c=3)
    ws3 = h["WS"][:].rearrange("p (c s) -> p c s", c=3)
    eng.tensor_tensor(ppn[:, 0:9 * Sh], ppn[:, 0:9 * Sh], ppn[:, 9 * Sh:],
                      OP.subtract)
    eng.tensor_tensor(ws3, ppv[:, :, 0], ppv[:, :, 1], OP.add)
    eng.tensor_tensor(w3, ws3, ppv[:, :, 2], OP.add)
    return w3


def _step_q_and_xq(eng, Sh, h, w3, xv2, Xn):
    """Q assembly from (w, alpha, beta planes in h["AB"]) and Xn = Xc @ Q."""
    ab2 = h["AB"][:].rearrange("p (g s) -> p g s", g=2)
    # [WP; WB] = [alpha*w ; beta*w] in one instruction
    wpb = h["WPB"][:].rearrange("p (g c s) -> p g c s", g=2, c=3)
    eng.tensor_tensor(
        wpb,
        w3.unsqueeze(1).broadcast_to((P, 2, 3, Sh)),
        ab2.unsqueeze(2).broadcast_to((P, 2, 3, Sh)),
        OP.mult)
    wpv = h["WPB"][:, 0:3 * Sh].rearrange("p (c s) -> p c s", c=3)
    wb3 = h["WPB"][:, 3 * Sh:].rearrange("p (c s) -> p c s", c=3)
    # Q = (beta*w) (x) w  +  alpha*I  +  skew(alpha*w)
    qv9 = h["QT"][:].rearrange("p (e s) -> p e s", e=9)
    qve = h["QT"][:].rearrange("p (a b s) -> p a b s", a=3, b=3)
    eng.tensor_tensor(qve,
                      wb3.unsqueeze(2).broadcast_to((P, 3, 3, Sh)),
                      w3.unsqueeze(1).broadcast_to((P, 3, 3, Sh)),
                      OP.mult)
    albc = h["AB"][:, 0:Sh].unsqueeze(1).broadcast_to((P, 3, Sh))
    eng.tensor_tensor(qv9[:, 0:9:4], qv9[:, 0:9:4], albc, OP.add)
    eng.tensor_tensor(qv9[:, 2:4], qv9[:, 2:4], wpv[:, 1:3], OP.add)
    eng.tensor_tensor(qv9[:, 7:8], qv9[:, 7:8], wpv[:, 0:1], OP.add)
    eng.tensor_tensor(qv9[:, 5:7], qv9[:, 5:7], wpv[:, 0:2], OP.subtract)
    eng.tensor_tensor(qv9[:, 1:2], qv9[:, 1:2], wpv[:, 2:3], OP.subtract)
    # Xn = Xc @ Q
    qv = h["QT"][:].rearrange("p (cc j s) -> p cc j s", cc=3, j=3)
    tbf = h["TBIG"]
    for cc in range(3):
        tv = tbf[:, cc * 9 * Sh:(cc + 1) * 9 * Sh].rearrange(
            "p (rr j s) -> p rr j s", rr=3, j=3)
        eng.tensor_tensor(
            tv,
            xv2[:, cc].unsqueeze(2).broadcast_to((P, 3, 3, Sh)),
            qv[:, cc].unsqueeze(1).broadcast_to((P, 3, 3, Sh)),
            OP.mult)
    eng.tensor_tensor(Xn[:], tbf[:, 0:9 * Sh], tbf[:, 9 * Sh:18 * Sh], OP.add)
    eng.tensor_tensor(Xn[:], Xn[:], tbf[:, 18 * Sh:], OP.add)


def build_nc(bl: int = BL, steps: int = STEPS, sb: int = SB_GPSIMD) -> bass.Bass:
    S = bl // P               # samples per partition
    F9 = 9 * S
    if sb * 2 >= S:
        sb = (S // 4) & ~1    # keep the split sane for small test sizes
    sd = S - sb               # DVE cohort width

    a0_, a1_, a2, a3 = ALPHA_POLY_B
    b1, b0 = BETA_A_POLY[1], BETA_A_POLY[0]
    nr_c0, nr_c1, nr_c2 = ALPHA_NR

    nc = bacc.Bacc("TRN2", target_bir_lowering=False, debug=False)
    with tile.TileContext(nc) as tc:
        x_d = nc.dram_tensor("x", [bl, 3, 3], F32, kind="ExternalInput")
        t_d = nc.dram_tensor("t", [bl, 1], F32, kind="ExternalInput")
        n_d = nc.dram_tensor("noise", [steps, bl, 3, 3], F32,
                             kind="ExternalInput")
        o_d = nc.dram_tensor("out", [bl, 3, 3], F32, kind="ExternalOutput")

        xr = x_d.rearrange("(p s) a b -> p (s a b)", p=P)
        tr = t_d.rearrange("(p s) o -> p (s o)", p=P)
        nr = n_d.rearrange("k (p s) a b -> k p (s a b)", p=P)
        orr = o_d.rearrange("(p s) a b -> p (s a b)", p=P)

        with (
            tc.tile_pool(name="state", bufs=1) as pool,
            tc.tile_pool(name="nzf", bufs=4) as nzfpool,
            tc.tile_pool(name="nza", bufs=3) as nzapool,
        ):
            XIN = pool.tile([P, F9], F32, name="XIN", tag="XIN")
            Tt = pool.tile([P, S], F32, name="Tt", tag="Tt")
            SD2H = pool.tile([P, S], F16, name="SD2H", tag="SD2H")
            SD2F = pool.tile([P, S], F32, name="SD2F", tag="SD2F")
            INVF = pool.tile([P, S], F32, name="INVF", tag="INVF")
            INVH = pool.tile([P, sd], F16, name="INVH", tag="INVH")
            OUTF = pool.tile([P, F9], F32, name="OUTF", tag="OUTF")

            # t first (tiny, unblocks sd2), then x, then the noise stream -
            # all on the same queue so the state DMAs are not stuck behind
            # noise prefetches
            nc.sync.dma_start(Tt[:], tr)
            NZF0 = nzfpool.tile([P, F9], F32, name="NZF", tag="NZF")
            sd_h = (sd // 2) & ~1
            nc.sync.dma_start(NZF0[:, 0:9 * sd], nr[0][:, 0:9 * sd])
            nc.sync.dma_start(XIN[:, 0:9 * sd_h], xr[:, 0:9 * sd_h])
            nc.sync.dma_start(XIN[:, 9 * sd_h:9 * sd], xr[:, 9 * sd_h:9 * sd])
            nc.sync.dma_start(XIN[:, 9 * sd:], xr[:, 9 * sd:])
            nc.sync.dma_start(NZF0[:, 9 * sd:], nr[0][:, 9 * sd:])
            # sd2 = 0.5*sqrt(t/steps) = sqrt(t/(4*steps))
            nc.scalar.activation(SD2H[:], Tt[:], AF.Sqrt, bias=0.0,
                                 scale=1.0 / (4.0 * steps))
            nc.scalar.activation(SD2F[:], Tt[:], AF.Sqrt, bias=0.0,
                                 scale=1.0 / (4.0 * steps))
            # 1/sd2 via the ~51-ULP fast reciprocal; emitted first in the DVE
            # program so it runs during the otherwise-idle ramp
            nc.vector.reciprocal_approx_fast(INVF[:], SD2F[:])

            # --- DVE cohort tiles (f16) ---
            ha = {
                "X": [pool.tile([P, 9 * sd], F16, name="XAa", tag="XAa"),
                      pool.tile([P, 9 * sd], F16, name="XAb", tag="XAb")],
                "PPN": pool.tile([P, 18 * sd], F16, name="PPNA", tag="PPNA"),
                "W": pool.tile([P, 3 * sd], F16, name="WA", tag="WA"),
                "WS": pool.tile([P, 3 * sd], F16, name="WSA", tag="WSA"),
                "TH2P": pool.tile([P, sd], F32, name="TH2PA", tag="TH2PA"),
                "U1": pool.tile([P, sd], F32, name="U1A", tag="U1A"),
                "AB": pool.tile([P, 2 * sd], F16, name="ABA", tag="ABA"),
                "WPB": pool.tile([P, 6 * sd], F16, name="WPBA", tag="WPBA"),
                "QT": pool.tile([P, 9 * sd], F16, name="QTA", tag="QTA"),
                "TBIG": pool.tile([P, 27 * sd], F16, name="TBIGA",
                                  tag="TBIGA"),
            }
            # --- GPSIMD cohort tiles (f32) ---
            hb = {
                "X": [pool.tile([P, 9 * sb], F32, name="XBa", tag="XBa"),
                      pool.tile([P, 9 * sb], F32, name="XBb", tag="XBb")],
                "PPN": pool.tile([P, 18 * sb], F32, name="PPNB", tag="PPNB"),
                "W": pool.tile([P, 3 * sb], F32, name="WB_", tag="WB_"),
                "WS": pool.tile([P, 3 * sb], F32, name="WSB", tag="WSB"),
                "P2": pool.tile([P, 3 * sb], F32, name="P2B", tag="P2B"),
                "TH2": pool.tile([P, sb], F32, name="TH2B", tag="TH2B"),
                "PH": pool.tile([P, sb], F32, name="PHB", tag="PHB"),
                "PH2": pool.tile([P, sb], F32, name="PHB2", tag="PHB2"),
                "U2": pool.tile([P, sb], F32, name="U2B", tag="U2B"),
                "AB": pool.tile([P, 2 * sb], F32, name="ABB", tag="ABB"),
                "WPB": pool.tile([P, 6 * sb], F32, name="WPBB", tag="WPBB"),
                "QT": pool.tile([P, 9 * sb], F32, name="QTB", tag="QTB"),
                "TBIG": pool.tile([P, 27 * sb], F32, name="TBIGB",
                                  tag="TBIGB"),
            }

            # initial states: X0 = sd2 * x straight from the AoS f32 input
            # (single TT per cohort; AoS strides cost nothing one-time)
            for i0, i1 in ((0, sd_h), (sd_h, sd)):
                xin_va = XIN[:, 9 * i0:9 * i1].rearrange(
                    "p (s e) -> p e s", e=9)
                nc.vector.tensor_tensor(
                    ha["X"][0][:].rearrange(
                        "p (e s) -> p e s", e=9)[:, :, i0:i1],
                    xin_va,
                    SD2H[:, i0:i1].unsqueeze(1).broadcast_to(
                        (P, 9, i1 - i0)), OP.mult)
            xin_vb = XIN[:, 9 * sd:].rearrange("p (s e) -> p e s", e=9)
            nc.gpsimd.tensor_tensor(
                hb["X"][0][:].rearrange("p (e s) -> p e s", e=9),
                xin_vb,
                SD2F[:, sd:].unsqueeze(1).broadcast_to((P, 9, sb)), OP.mult)

            for k in range(steps):
                if k == 0:
                    NZF = NZF0
                else:
                    NZF = nzfpool.tile([P, F9], F32, name="NZF", tag="NZF")
                    nc.sync.dma_start(NZF[:], nr[k])
                # DVE cohort noise: AoS f32 -> SoA f16 (ScalarE)
                NZA = nzapool.tile([P, 9 * sd], F16, name="NZA", tag="NZA")
                nzf_va = NZF[:, 0:9 * sd].rearrange("p (s e) -> p e s", e=9)
                nc.scalar.copy(NZA[:].rearrange("p (e s) -> p e s", e=9),
                               nzf_va)

                # ---------- DVE cohort ----------
                eng = nc.vector
                Xc, Xn = ha["X"][k % 2], ha["X"][(k + 1) % 2]
                xv2 = Xc[:].rearrange("p (rr e s) -> p e rr s", rr=3, e=3)
                nv2 = NZA[:].rearrange("p (rr e s) -> p e rr s", rr=3, e=3)
                w3 = _step_common(eng, sd, ha, xv2, nv2, True)
                # theta^2 and alpha via fused custom ops, beta via one TSP
                eng._custom_dve(SQSQ, out=ha["TH2P"][:],
                                in0=ha["W"][:, 0:sd], in1=ha["W"][:, sd:2 * sd])
                eng._custom_dve(SQADD1P, out=ha["U1"][:],
                                in0=ha["W"][:, 2 * sd:], in1=ha["TH2P"][:])
                eng._custom_dve(ALPHA_FULL, out=ha["AB"][:, 0:sd],
                                in0=ha["U1"][:], s0=nr_c0, s1=nr_c1,
                                imm2=nr_c2)
                eng.tensor_scalar(ha["AB"][:, sd:], ha["AB"][:, 0:sd],
                                  float(b1), float(b0), OP.mult, OP.add)
                _step_q_and_xq(eng, sd, ha, w3, xv2, Xn)

                # ---------- GPSIMD cohort (f32, raw AoS noise) ----------
                eng = nc.gpsimd
                Xc, Xn = hb["X"][k % 2], hb["X"][(k + 1) % 2]
                xv2b = Xc[:].rearrange("p (rr e s) -> p e rr s", rr=3, e=3)
                nv2b = NZF[:, 9 * sd:].rearrange("p (s rr e) -> p e rr s",
                                                 rr=3, e=3)
                w3b = _step_common(eng, sb, hb, xv2b, nv2b, False)
                # theta^2
                p2v = hb["P2"][:].rearrange("p (c s) -> p c s", c=3)
                eng.tensor_tensor(hb["P2"][:], hb["W"][:], hb["W"][:],
                                  OP.mult)
                eng.tensor_tensor(hb["TH2"][:], p2v[:, 0], p2v[:, 1], OP.add)
                eng.tensor_tensor(hb["TH2"][:], hb["TH2"][:], p2v[:, 2],
                                  OP.add)
                # alpha deg-3 in Estrin form (three leading ops independent)
                eng.tensor_scalar(hb["PH"][:], hb["TH2"][:], float(a1_),
                                  float(a0_), OP.mult, OP.add)
                eng.tensor_scalar(hb["PH2"][:], hb["TH2"][:], float(a3),
                                  float(a2), OP.mult, OP.add)
                eng.tensor_tensor(hb["U2"][:], hb["TH2"][:], hb["TH2"][:],
                                  OP.mult)
                eng.tensor_tensor(hb["PH2"][:], hb["PH2"][:], hb["U2"][:],
                                  OP.mult)
                eng.tensor_tensor(hb["AB"][:, 0:sb], hb["PH"][:], hb["PH2"][:],
                                  OP.add)
                eng.tensor_scalar(hb["AB"][:, sb:], hb["AB"][:, 0:sb],
                                  float(b1), float(b0), OP.mult, OP.add)
                _step_q_and_xq(eng, sb, hb, w3b, xv2b, Xn)

            # final: unscale (x = Xs / sd2), convert to AoS f32, DMA out
            nc.scalar.copy(INVH[:], INVF[:, 0:sd])
            xfa = ha["X"][steps % 2]
            XOUT = pool.tile([P, 9 * sd], F16, name="XOUT", tag="XOUT")
            sd_3 = (sd // 3) & ~1
            for c0, c1 in ((0, sd_3), (sd_3, 2 * sd_3), (2 * sd_3, sd)):
                cw = c1 - c0
                nc.vector.tensor_tensor(
                    XOUT[:, 9 * c0:9 * c1].rearrange("p (e s) -> p e s", e=9),
                    xfa[:].rearrange("p (e s) -> p e s", e=9)[:, :, c0:c1],
                    INVH[:, c0:c1].unsqueeze(1).broadcast_to((P, 9, cw)),
                    OP.mult)
                of_va = OUTF[:, 9 * c0:9 * c1].rearrange(
                    "p (s e) -> p s e", e=9)
                nc.scalar.copy(of_va, XOUT[:, 9 * c0:9 * c1].rearrange(
                    "p (e s) -> p s e", e=9))
                nc.scalar.dma_start(orr[:, 9 * c0:9 * c1],
                                    OUTF[:, 9 * c0:9 * c1])
            # GPSIMD cohort writes its AoS f32 slice directly
            xfb = hb["X"][steps % 2]
            of_vb = OUTF[:, 9 * sd:].rearrange("p (s e) -> p e s", e=9)
            nc.gpsimd.tensor_tensor(
                of_vb,
                xfb[:].rearrange("p (e s) -> p e s", e=9),
                INVF[:, sd:].unsqueeze(1).broadcast_to((P, 9, sb)), OP.mult)
            nc.sync.dma_start(orr[:, 9 * sd:], OUTF[:, 9 * sd:])
    nc.compile()
    return nc


_NC_CACHE = {}


def _get_nc(bl: int, steps: int) -> bass.Bass:
    key = (bl, steps)
    if key not in _NC_CACHE:
        _NC_CACHE[key] = build_nc(bl, steps)
    return _NC_CACHE[key]


last_exec_time_ns = None
last_results = None


def kernel(x: np.ndarray, t: np.ndarray, noise: np.ndarray, steps=STEPS,
           _trace: bool = False, **_unused) -> np.ndarray:
    global last_exec_time_ns, last_results
    steps = int(steps)
    b = x.shape[0]
    assert b % NCORES == 0
    bl = b // NCORES
    assert bl % P == 0

    x = np.ascontiguousarray(np.asarray(x, dtype=np.float32))
    t = np.ascontiguousarray(np.asarray(t, dtype=np.float32))
    noise = np.ascontiguousarray(np.asarray(noise, dtype=np.float32))

    nc = _get_nc(bl, steps)
    in_maps = []
    for i in range(NCORES):
        sl = slice(i * bl, (i + 1) * bl)
        in_maps.append({
            "x": x[sl],
            "t": t[sl],
            "noise": np.ascontiguousarray(noise[:, sl]),
        })
    res = run_bass_kernel_spmd(
        nc, in_maps, core_ids=list(range(NCORES)), trace=_trace)
    last_exec_time_ns = res.exec_time_ns
    last_results = res
    out = np.concatenate([r["out"] for r in res.results], axis=0)
    return out.astype(np.float32)


# revision 22
# speedup vs baseline: 1.0057x; 1.0018x over previous
"""Trainium2 Bass kernel: Brownian motion on O(3) via ambient SDE steps.

Math: each reference step is
    inc = sqrt(dt) * eps
    v   = 0.5*(inc - x inc^T x) = x @ Omega,  Omega = 0.5*(A - A^T), A = x^T inc
    x'  = polar(x + v) = x @ polar(I + Omega)
and for a 3x3 skew Omega with axis vector w (|w| = theta):
    polar(I + Omega) = Q = alpha*I + Omega(alpha*w) + beta * w w^T
    c = sqrt(1 + theta^2), alpha = 1/c, beta = 1/(c*(c+1))
which matches the SVD projection to machine precision (no SVD needed).

The per-sample scale sd2 = 0.5*sqrt(t/steps) is folded into the state:
    Xs = sd2 * x   =>   w = axial(Xs^T N) needs no per-step scaling, and
    Xs' = Xs @ Q keeps the fold;  x_final = Xs_final / sd2  (once).

Two decoupled cohorts per core (samples split along the free dim):
  - DVE cohort (fp16 SoA planes, 2x_1p mode): products/assembly/xQ as
    tensor_tensor; theta^2 and alpha = rsqrt(1+theta^2) via three custom
    DVE ops (sq(a)+sq(b); sq(a)+b+1; linear-seed + one Newton step fused,
    max rel err 2.7e-4 on theta^2 in [0, 0.9]); beta(alpha) deg-1 via one
    fused tensor_scalar.
  - GPSIMD cohort (fp32): reads the raw AoS f32 noise directly (Pool has
    no stride-1 perf mode, so AoS strides are free) - no ScalarE
    conversion, no cross-engine deps at all; alpha poly deg-3 via fused
    immediate tensor_scalar ops (Pool-legal, and 0.6 ISA efficiency vs
    0.42 for tensor_tensor).
ScalarE only converts the DVE cohort's noise (AoS f32 -> SoA f16) and
does one-time init/final layout conversions. Steady state has no
cross-engine round trips inside a step, so both engines run at their
own throughput; the sample split balances them.

Sharding: pure data parallel over the batch across 8 NeuronCores.
"""

import os
import sys

import numpy as np

for _p in ("/opt/trn_rl_repo",):
    if _p not in sys.path and os.path.isdir(_p):
        sys.path.insert(0, _p)

import concourse.bass as bass
import concourse.dve_ops as dve_ops
import concourse.tile as tile
from concourse import bacc, mybir
from concourse.bass_utils import run_bass_kernel_spmd
from concourse.dve_ops import DveOp
from concourse.dve_spec import C0, C1, C2, One, Spec, Src0, Src1, lower, sq
from concourse.dve_table_gen import dve_ver_for
from concourse.dve_uop import DveOpSpec

AF = mybir.ActivationFunctionType
OP = mybir.AluOpType
F32 = mybir.dt.float32
F16 = mybir.dt.float16

B = 262144
NCORES = 8
BL = B // NCORES          # 32768 samples per core
P = 128
STEPS = 20

# samples per partition handled by GPSIMD (rest on DVE)
SB_GPSIMD = 49

# alpha = rsqrt(1 + theta^2): linear seed + one fused Newton step, constants
# minimax-optimized jointly on u1 = 1+theta^2 in [1, 1.9] (rel err 2.74e-4)
ALPHA_NR = (1.26296369, -0.29519499, 0.52064811)
# beta as a degree-1 polynomial OF ALPHA (beta = a^2/(1+a)); the ~5e-3 fit
# error is damped by theta^2 in Q's rank-1 term
BETA_A_POLY = [-0.20798077392841205, 0.705238169782092]
# GPSIMD cohort: alpha(u) deg-3 minimax fit on [0, 0.9] (3.5e-4), evaluated
# in Estrin form so the two halves overlap (fewer serial-dependency gaps)
ALPHA_POLY_B = [0.99964635, -0.48630541, 0.28591126, -0.09374474]


def _register_op(name, spec, rd1: bool) -> DveOp:
    """Register a custom DVE op (idempotent across re-imports)."""
    for op in dve_ops.OPS:
        if op.name == name:
            return op
    ver = dve_ver_for("TRN2")
    row = dve_ops._CUSTOM_DVE_ROW_BASE + len(dve_ops.OPS)
    sha = DveOpSpec(name=name, opcode=row, uops=lower(spec, ver=ver),
                    rd1_en=rd1).sha(ver)
    op = DveOp(name, spec, subdim=False, uops_sha={ver: sha})
    dve_ops.OPS.append(op)
    dve_ops._SUB_OPCODE_FOR_NAME[name] = row
    dve_ops.CUSTOM_DVE_SPECS[name] = spec
    return op


# th2 partial: out = w0^2 + w1^2
SQSQ = _register_op("BM_SQSQ", Spec(
    body=sq(Src0) + sq(Src1),
    reference=lambda in0, in1, s0, s1, imm2: (
        in0.astype(np.float32) ** 2 + in1.astype(np.float32) ** 2),
), rd1=True)
# u1 = 1 + w2^2 + partial  (= 1 + theta^2)
SQADD1P = _register_op("BM_SQADD1P", Spec(
    body=sq(Src0) + Src1 + One,
    reference=lambda in0, in1, s0, s1, imm2: (
        in0.astype(np.float32) ** 2 + in1.astype(np.float32) + 1.0),
), rd1=True)


def _alpha_ref(in0, in1, s0, s1, imm2):
    x = in0.astype(np.float32)
    a0 = x * s1 + s0
    return a0 * (1.0 + imm2 * (1.0 - a0 * a0 * x))


# alpha = rsqrt(u1): a0 = C0 + C1*u1 (seed), out = a0*(1 + C2*(1 - u1*a0^2))
_a0 = Src0 * C1 + C0
ALPHA_FULL = _register_op("BM_ALPHA_FULL", Spec(
    body=_a0 * (One + C2 * (One - sq(_a0) * Src0)),
    reference=_alpha_ref,
), rd1=False)


def _step_common(eng, Sh, h, xv2, nv2, f16: bool):
    """Products -> w -> (theta^2 path is per-engine) shared plumbing.
    Emits products + skew-difference + reduction into h["W"]."""
    ppn = h["PPN"]
    ppv = ppn[:, 0:9 * Sh].rearrange("p (c r s) -> p c r s", c=3, r=3)
    pnv = ppn[:, 9 * Sh:].rearrange("p (c r s) -> p c r s", c=3, r=3)
    gv = ppn[:].rearrange("p (g r s) -> p g r s", g=6, r=3)
    # merged product instructions (3 instead of 6)
    eng.tensor_tensor(ppv[:, 1:3], xv2[:, 0:2], nv2[:, 2::-2], OP.mult)
    eng.tensor_tensor(pnv[:, 0:2], xv2[:, 1:3], nv2[:, 2::-2], OP.mult)
    eng.tensor_tensor(gv[:, 0:6:5], xv2[:, 2::-2],
                      nv2[:, 1:2].broadcast_to((P, 2, 3, Sh)), OP.mult)
    # w_c = sum_r (PP - PN)   (sd2 already folded into X)
    w3 = h["W"][:].rearrange("p (c s) -> p c s", c=3)
    ws3 = h["WS"][:].rearrange("p (c s) -> p c s", c=3)
    eng.tensor_tensor(ppn[:, 0:9 * Sh], ppn[:, 0:9 * Sh], ppn[:, 9 * Sh:],
                      OP.subtract)
    eng.tensor_tensor(ws3, ppv[:, :, 0], ppv[:, :, 1], OP.add)
    eng.tensor_tensor(w3, ws3, ppv[:, :, 2], OP.add)
    return w3


def _step_q_and_xq(eng, Sh, h, w3, xv2, Xn):
    """Q assembly from (w, alpha, beta planes in h["AB"]) and Xn = Xc @ Q."""
    ab2 = h["AB"][:].rearrange("p (g s) -> p g s", g=2)
    # [WP; WB] = [alpha*w ; beta*w] in one instruction
    wpb = h["WPB"][:].rearrange("p (g c s) -> p g c s", g=2, c=3)
    eng.tensor_tensor(
        wpb,
        w3.unsqueeze(1).broadcast_to((P, 2, 3, Sh)),
        ab2.unsqueeze(2).broadcast_to((P, 2, 3, Sh)),
        OP.mult)
    wpv = h["WPB"][:, 0:3 * Sh].rearrange("p (c s) -> p c s", c=3)
    wb3 = h["WPB"][:, 3 * Sh:].rearrange("p (c s) -> p c s", c=3)
    # Q = (beta*w) (x) w  +  alpha*I  +  skew(alpha*w)
    qv9 = h["QT"][:].rearrange("p (e s) -> p e s", e=9)
    qve = h["QT"][:].rearrange("p (a b s) -> p a b s", a=3, b=3)
    eng.tensor_tensor(qve,
                      wb3.unsqueeze(2).broadcast_to((P, 3, 3, Sh)),
                      w3.unsqueeze(1).broadcast_to((P, 3, 3, Sh)),
                      OP.mult)
    albc = h["AB"][:, 0:Sh].unsqueeze(1).broadcast_to((P, 3, Sh))
    eng.tensor_tensor(qv9[:, 0:9:4], qv9[:, 0:9:4], albc, OP.add)
    eng.tensor_tensor(qv9[:, 2:4], qv9[:, 2:4], wpv[:, 1:3], OP.add)
    eng.tensor_tensor(qv9[:, 7:8], qv9[:, 7:8], wpv[:, 0:1], OP.add)
    eng.tensor_tensor(qv9[:, 5:7], qv9[:, 5:7], wpv[:, 0:2], OP.subtract)
    eng.tensor_tensor(qv9[:, 1:2], qv9[:, 1:2], wpv[:, 2:3], OP.subtract)
    # Xn = Xc @ Q
    qv = h["QT"][:].rearrange("p (cc j s) -> p cc j s", cc=3, j=3)
    tbf = h["TBIG"]
    for cc in range(3):
        tv = tbf[:, cc * 9 * Sh:(cc + 1) * 9 * Sh].rearrange(
            "p (rr j s) -> p rr j s", rr=3, j=3)
        eng.tensor_tensor(
            tv,
            xv2[:, cc].unsqueeze(2).broadcast_to((P, 3, 3, Sh)),
            qv[:, cc].unsqueeze(1).broadcast_to((P, 3, 3, Sh)),
            OP.mult)
    eng.tensor_tensor(Xn[:], tbf[:, 0:9 * Sh], tbf[:, 9 * Sh:18 * Sh], OP.add)
    eng.tensor_tensor(Xn[:], Xn[:], tbf[:, 18 * Sh:], OP.add)


def build_nc(bl: int = BL, steps: int = STEPS, sb: int = SB_GPSIMD) -> bass.Bass:
    S = bl // P               # samples per partition
    F9 = 9 * S
    if sb * 2 >= S:
        sb = (S // 4) & ~1    # keep the split sane for small test sizes
    sd = S - sb               # DVE cohort width

    a0_, a1_, a2, a3 = ALPHA_POLY_B
    b1, b0 = BETA_A_POLY[1], BETA_A_POLY[0]
    nr_c0, nr_c1, nr_c2 = ALPHA_NR

    nc = bacc.Bacc("TRN2", target_bir_lowering=False, debug=False)
    with tile.TileContext(nc) as tc:
        x_d = nc.dram_tensor("x", [bl, 3, 3], F32, kind="ExternalInput")
        t_d = nc.dram_tensor("t", [bl, 1], F32, kind="ExternalInput")
        n_d = nc.dram_tensor("noise", [steps, bl, 3, 3], F32,
                             kind="ExternalInput")
        o_d = nc.dram_tensor("out", [bl, 3, 3], F32, kind="ExternalOutput")

        xr = x_d.rearrange("(p s) a b -> p (s a b)", p=P)
        tr = t_d.rearrange("(p s) o -> p (s o)", p=P)
        nr = n_d.rearrange("k (p s) a b -> k p (s a b)", p=P)
        orr = o_d.rearrange("(p s) a b -> p (s a b)", p=P)

        with (
            tc.tile_pool(name="state", bufs=1) as pool,
            tc.tile_pool(name="nzf", bufs=4) as nzfpool,
            tc.tile_pool(name="nza", bufs=3) as nzapool,
        ):
            XIN = pool.tile([P, F9], F32, name="XIN", tag="XIN")
            Tt = pool.tile([P, S], F32, name="Tt", tag="Tt")
            SD2H = pool.tile([P, S], F16, name="SD2H", tag="SD2H")
            SD2F = pool.tile([P, S], F32, name="SD2F", tag="SD2F")
            INVF = pool.tile([P, S], F32, name="INVF", tag="INVF")
            INVH = pool.tile([P, sd], F16, name="INVH", tag="INVH")
            OUTF = pool.tile([P, F9], F32, name="OUTF", tag="OUTF")

            # t first (tiny, unblocks sd2), then x, then the noise stream -
            # all on the same queue so the state DMAs are not stuck behind
            # noise prefetches
            nc.sync.dma_start(Tt[:], tr)
            NZF0 = nzfpool.tile([P, F9], F32, name="NZF", tag="NZF")
            sd_h = (sd // 2) & ~1
            nc.sync.dma_start(NZF0[:, 0:9 * sd], nr[0][:, 0:9 * sd])
            nc.sync.dma_start(XIN[:, 0:9 * sd_h], xr[:, 0:9 * sd_h])
            nc.sync.dma_start(XIN[:, 9 * sd_h:9 * sd], xr[:, 9 * sd_h:9 * sd])
            nc.sync.dma_start(XIN[:, 9 * sd:], xr[:, 9 * sd:])
            nc.sync.dma_start(NZF0[:, 9 * sd:], nr[0][:, 9 * sd:])
            # sd2 = 0.5*sqrt(t/steps) = sqrt(t/(4*steps))
            nc.scalar.activation(SD2H[:], Tt[:], AF.Sqrt, bias=0.0,
                                 scale=1.0 / (4.0 * steps))
            nc.scalar.activation(SD2F[:], Tt[:], AF.Sqrt, bias=0.0,
                                 scale=1.0 / (4.0 * steps))
            # 1/sd2 via the ~51-ULP fast reciprocal; emitted first in the DVE
            # program so it runs during the otherwise-idle ramp
            nc.vector.reciprocal_approx_fast(INVF[:], SD2F[:])

            # --- DVE cohort tiles (f16) ---
            ha = {
                "X": [pool.tile([P, 9 * sd], F16, name="XAa", tag="XAa"),
                      pool.tile([P, 9 * sd], F16, name="XAb", tag="XAb")],
                "PPN": pool.tile([P, 18 * sd], F16, name="PPNA", tag="PPNA"),
                "W": pool.tile([P, 3 * sd], F16, name="WA", tag="WA"),
                "WS": pool.tile([P, 3 * sd], F16, name="WSA", tag="WSA"),
                "TH2P": pool.tile([P, sd], F32, name="TH2PA", tag="TH2PA"),
                "U1": pool.tile([P, sd], F32, name="U1A", tag="U1A"),
                "AB": pool.tile([P, 2 * sd], F16, name="ABA", tag="ABA"),
                "WPB": pool.tile([P, 6 * sd], F16, name="WPBA", tag="WPBA"),
                "QT": pool.tile([P, 9 * sd], F16, name="QTA", tag="QTA"),
                "TBIG": pool.tile([P, 27 * sd], F16, name="TBIGA",
                                  tag="TBIGA"),
            }
            # --- GPSIMD cohort tiles (f32) ---
            hb = {
                "X": [pool.tile([P, 9 * sb], F32, name="XBa", tag="XBa"),
                      pool.tile([P, 9 * sb], F32, name="XBb", tag="XBb")],
                "PPN": pool.tile([P, 18 * sb], F32, name="PPNB", tag="PPNB"),
                "W": pool.tile([P, 3 * sb], F32, name="WB_", tag="WB_"),
                "WS": pool.tile([P, 3 * sb], F32, name="WSB", tag="WSB"),
                "P2": pool.tile([P, 3 * sb], F32, name="P2B", tag="P2B"),
                "TH2": pool.tile([P, sb], F32, name="TH2B", tag="TH2B"),
                "PH": pool.tile([P, sb], F32, name="PHB", tag="PHB"),
                "PH2": pool.tile([P, sb], F32, name="PHB2", tag="PHB2"),
                "U2": pool.tile([P, sb], F32, name="U2B", tag="U2B"),
                "AB": pool.tile([P, 2 * sb], F32, name="ABB", tag="ABB"),
                "WPB": pool.tile([P, 6 * sb], F32, name="WPBB", tag="WPBB"),
                "QT": pool.tile([P, 9 * sb], F32, name="QTB", tag="QTB"),
                "TBIG": pool.tile([P, 27 * sb], F32, name="TBIGB",
                                  tag="TBIGB"),
            }

            # initial states: X0 = sd2 * x straight from the AoS f32 input
            # (single TT per cohort; AoS strides cost nothing one-time)
            for i0, i1 in ((0, sd_h), (sd_h, sd)):
                xin_va = XIN[:, 9 * i0:9 * i1].rearrange(
                    "p (s e) -> p e s", e=9)
                nc.vector.tensor_tensor(
                    ha["X"][0][:].rearrange(
                        "p (e s) -> p e s", e=9)[:, :, i0:i1],
                    xin_va,
                    SD2H[:, i0:i1].unsqueeze(1).broadcast_to(
                        (P, 9, i1 - i0)), OP.mult)
            xin_vb = XIN[:, 9 * sd:].rearrange("p (s e) -> p e s", e=9)
            nc.gpsimd.tensor_tensor(
                hb["X"][0][:].rearrange("p (e s) -> p e s", e=9),
                xin_vb,
                SD2F[:, sd:].unsqueeze(1).broadcast_to((P, 9, sb)), OP.mult)

            for k in range(steps):
                if k == 0:
                    NZF = NZF0
                else:
                    NZF = nzfpool.tile([P, F9], F32, name="NZF", tag="NZF")
                    nc.sync.dma_start(NZF[:], nr[k])
                # DVE cohort noise: AoS f32 -> SoA f16 (ScalarE)
                NZA = nzapool.tile([P, 9 * sd], F16, name="NZA", tag="NZA")
                nzf_va = NZF[:, 0:9 * sd].rearrange("p (s e) -> p e s", e=9)
                nc.scalar.copy(NZA[:].rearrange("p (e s) -> p e s", e=9),
                               nzf_va)

                # ---------- DVE cohort ----------
                eng = nc.vector
                Xc, Xn = ha["X"][k % 2], ha["X"][(k + 1) % 2]
                xv2 = Xc[:].rearrange("p (rr e s) -> p e rr s", rr=3, e=3)
                nv2 = NZA[:].rearrange("p (rr e s) -> p e rr s", rr=3, e=3)
                w3 = _step_common(eng, sd, ha, xv2, nv2, True)
                # theta^2 and alpha via fused custom ops, beta via one TSP
                eng._custom_dve(SQSQ, out=ha["TH2P"][:],
                                in0=ha["W"][:, 0:sd], in1=ha["W"][:, sd:2 * sd])
                eng._custom_dve(SQADD1P, out=ha["U1"][:],
                                in0=ha["W"][:, 2 * sd:], in1=ha["TH2P"][:])
                eng._custom_dve(ALPHA_FULL, out=ha["AB"][:, 0:sd],
                                in0=ha["U1"][:], s0=nr_c0, s1=nr_c1,
                                imm2=nr_c2)
                eng.tensor_scalar(ha["AB"][:, sd:], ha["AB"][:, 0:sd],
                                  float(b1), float(b0), OP.mult, OP.add)
                _step_q_and_xq(eng, sd, ha, w3, xv2, Xn)

                # ---------- GPSIMD cohort (f32, raw AoS noise) ----------
                eng = nc.gpsimd
                Xc, Xn = hb["X"][k % 2], hb["X"][(k + 1) % 2]
                xv2b = Xc[:].rearrange("p (rr e s) -> p e rr s", rr=3, e=3)
                nv2b = NZF[:, 9 * sd:].rearrange("p (s rr e) -> p e rr s",
                                                 rr=3, e=3)
                w3b = _step_common(eng, sb, hb, xv2b, nv2b, False)
                # theta^2
                p2v = hb["P2"][:].rearrange("p (c s) -> p c s", c=3)
                eng.tensor_tensor(hb["P2"][:], hb["W"][:], hb["W"][:],
                                  OP.mult)
                eng.tensor_tensor(hb["TH2"][:], p2v[:, 0], p2v[:, 1], OP.add)
                eng.tensor_tensor(hb["TH2"][:], hb["TH2"][:], p2v[:, 2],
                                  OP.add)
                # alpha deg-3 in Estrin form (three leading ops independent)
                eng.tensor_scalar(hb["PH"][:], hb["TH2"][:], float(a1_),
                                  float(a0_), OP.mult, OP.add)
                eng.tensor_scalar(hb["PH2"][:], hb["TH2"][:], float(a3),
                                  float(a2), OP.mult, OP.add)
                eng.tensor_tensor(hb["U2"][:], hb["TH2"][:], hb["TH2"][:],
                                  OP.mult)
                eng.tensor_tensor(hb["PH2"][:], hb["PH2"][:], hb["U2"][:],
                                  OP.mult)
                eng.tensor_tensor(hb["AB"][:, 0:sb], hb["PH"][:], hb["PH2"][:],
                                  OP.add)
                eng.tensor_scalar(hb["AB"][:, sb:], hb["AB"][:, 0:sb],
                                  float(b1), float(b0), OP.mult, OP.add)
                _step_q_and_xq(eng, sb, hb, w3b, xv2b, Xn)

            # final: unscale (x = Xs / sd2), convert to AoS f32, DMA out
            nc.scalar.copy(INVH[:], INVF[:, 0:sd])
            xfa = ha["X"][steps % 2]
            XOUT = pool.tile([P, 9 * sd], F16, name="XOUT", tag="XOUT")
            sd_3 = (sd // 3) & ~1
            chunks = ((0, sd_3), (sd_3, 2 * sd_3), (2 * sd_3, sd))
            for c0, c1 in chunks:
                cw = c1 - c0
                nc.vector.tensor_tensor(
                    XOUT[:, 9 * c0:9 * c1].rearrange("p (e s) -> p e s", e=9),
                    xfa[:].rearrange("p (e s) -> p e s", e=9)[:, :, c0:c1],
                    INVH[:, c0:c1].unsqueeze(1).broadcast_to((P, 9, cw)),
                    OP.mult)
                of_va = OUTF[:, 9 * c0:9 * c1].rearrange(
                    "p (s e) -> p s e", e=9)
                nc.scalar.copy(of_va, XOUT[:, 9 * c0:9 * c1].rearrange(
                    "p (e s) -> p s e", e=9))
            for c0, c1 in chunks:
                nc.scalar.dma_start(orr[:, 9 * c0:9 * c1],
                                    OUTF[:, 9 * c0:9 * c1])
            # GPSIMD cohort writes its AoS f32 slice directly
            xfb = hb["X"][steps % 2]
            of_vb = OUTF[:, 9 * sd:].rearrange("p (s e) -> p e s", e=9)
            nc.gpsimd.tensor_tensor(
                of_vb,
                xfb[:].rearrange("p (e s) -> p e s", e=9),
                INVF[:, sd:].unsqueeze(1).broadcast_to((P, 9, sb)), OP.mult)
            nc.sync.dma_start(orr[:, 9 * sd:], OUTF[:, 9 * sd:])
    nc.compile()
    return nc


_NC_CACHE = {}


def _get_nc(bl: int, steps: int) -> bass.Bass:
    key = (bl, steps)
    if key not in _NC_CACHE:
        _NC_CACHE[key] = build_nc(bl, steps)
    return _NC_CACHE[key]


last_exec_time_ns = None
last_results = None


def kernel(x: np.ndarray, t: np.ndarray, noise: np.ndarray, steps=STEPS,
           _trace: bool = False, **_unused) -> np.ndarray:
    global last_exec_time_ns, last_results
    steps = int(steps)
    b = x.shape[0]
    assert b % NCORES == 0
    bl = b // NCORES
    assert bl % P == 0

    x = np.ascontiguousarray(np.asarray(x, dtype=np.float32))
    t = np.ascontiguousarray(np.asarray(t, dtype=np.float32))
    noise = np.ascontiguousarray(np.asarray(noise, dtype=np.float32))

    nc = _get_nc(bl, steps)
    in_maps = []
    for i in range(NCORES):
        sl = slice(i * bl, (i + 1) * bl)
        in_maps.append({
            "x": x[sl],
            "t": t[sl],
            "noise": np.ascontiguousarray(noise[:, sl]),
        })
    res = run_bass_kernel_spmd(
        nc, in_maps, core_ids=list(range(NCORES)), trace=_trace)
    last_exec_time_ns = res.exec_time_ns
    last_results = res
    out = np.concatenate([r["out"] for r in res.results], axis=0)
    return out.astype(np.float32)
